# revision 23
# baseline (speedup 1.0000x reference)
"""Trainium2 Bass kernel for nn_Net_76270029242478 (gnn_message_passing).

Math (B=32, N=100, E=256, H=1024, MID=256):
  t        = einsum('bije,em->bijm', trans_mat, W_r) + b_r
  qp       = q @ W_q + b_q
  relation = einsum('bijm,m->bij', t * qp[:,None,None,:], W_out[:,0]) + b_out
  relation = where(r_mask==0, -inf, relation); softmax over i (axis=1)
  out      = einsum('bij,bj->bi', softmax, z_logits)

Algebraic fold (exact):
  relation[b,i,j] = trans_mat[b,i,j,:] . u[b,:] + c[b]
    u[b,e] = sum_m W_r[e,m] * (qp[b,m]+b_q[m]) * W_out[m,0]
  c[b] is constant over (i,j) so it cancels in the softmax over i.

Device strategy (v2 — bf16 stream + overlapped 128-col stationary windows):
  - The device matmul consumes bf16 anyway, so the host casts trans to bf16
    before staging: the HBM stream halves to 20.5 MB/core (~57 us at the
    ~358 GB/s per-core HBM roofline) vs 41 MB f32, and the on-device
    DVE/ACT cast pipeline disappears entirely.
  - Host lays trans out flat as [b, e, i*N+j] (j contiguous), zero-padded
    to NF columns, so every DMA is one fat contiguous run per partition.
  - The PE mat-vec rate in the f32 baseline was LDWEIGHTS-bound: 100-col
    stationary loads stream at 1 col / 1.2 GHz = 83 ns, 87 ns/pair
    measured, 70 us total — above the new DMA floor. Fix: stationary
    windows of 128 columns taken at stride 100 from the flat buffer
    (row i's 100 j-columns + 28 duplicate elements of row i+1). 128-col
    bf16 loads are eligible for Fast Weight Load (~2x), and psum
    partition p<100 of column t still receives exactly rel[j=p, i=t];
    partitions 100:128 hold duplicates that are never read.
  - Each DMA chunk carries 28 duplicate trailing columns so stationary
    windows never cross chunk-tile boundaries (+0.6% bytes).
  - The whole stream runs on the SWDGE (gpsimd) queue: splitting across
    a second HWDGE queue measured 250 GB/s aggregate vs ~420 single-queue.
    Consts ride the scalar HWDGE ring and must clear before the SWDGE
    queue saturates (starved HWDGE packets trickle at ~3 GB/s after that).
  - Softmax lands in [j_part, i_free] layout: exp (ACT), mask-mult +
    denominator (one DVE op with accum), final aggregation is one matmul.
  - The last sample's chunks taper so the post-stream drain is short.

Sharding: data-parallel over batch, 4 samples per core x 8 cores.
"""

import ml_dtypes
import numpy as np

import concourse.bass as bass
import concourse.tile as tile
from concourse import bacc, mybir
from concourse.bass_utils import run_bass_kernel_spmd

F32 = mybir.dt.float32
BF16 = mybir.dt.bfloat16
Alu = mybir.AluOpType
ActF = mybir.ActivationFunctionType

B, N, E, H, MID = 32, 100, 256, 1024, 256
NCORES = 8
BPC = B // NCORES       # samples per core = 4
EH = E // 128           # 2 e-halves (contraction chunks)
HK = H // 128           # 8 contraction chunks for q @ W_q
MK = MID // 128         # 2 contraction chunks
STAT = 128              # stationary window width (128 -> FWL-eligible)
OVL = STAT - N          # overlap columns carried past each chunk end
NF = N * N + 48         # flat (i,j) length, zero-padded
# i-row chunk schedule per sample: (row0, nrows). ~1.3 MB transfers keep
# the SWDGE descriptor pipeline dense (whole-sample 2.6 MB transfers delay
# the doorbell and sample-serialize the PE); sample 0 starts small so the
# PE gets work early, the last sample tapers so the post-stream drain is
# short.
CHUNKS = {
    0: [(0, 50), (50, 50)],
    1: [(0, 50), (50, 50)],
    2: [(0, 50), (50, 50)],
    3: [(0, 50), (50, 25), (75, 13), (88, 12)],
}
# packA (bf16) column offsets
A_WQ, A_WR, A_Q = 0, HK * MID, HK * MID + MK * E
A_W = A_Q + HK * BPC                     # 2592
# packB (f32) column offsets
B_BW, B_Z, B_MASK = 0, 2 * MK, 2 * MK + BPC
B_W = B_MASK + BPC * N                   # 408


def _build():
    nc = bacc.Bacc("TRN2", target_bir_lowering=False, debug=False,
                   num_devices=NCORES)

    # trans pre-flattened on host to [b, e, i*N+j] bf16 (e on partitions)
    transF_d = nc.declare_dram_parameter("transF", [BPC, E, NF], BF16,
                                         isOutput=False)
    packA_d = nc.declare_dram_parameter("packA", [128, A_W], BF16, isOutput=False)
    packB_d = nc.declare_dram_parameter("packB", [128, B_W], F32, isOutput=False)
    outT_d = nc.declare_dram_parameter("outT", [N, BPC], F32, isOutput=True)

    with tile.TileContext(nc) as tc, \
         tc.tile_pool(name="const", bufs=1) as const_pool, \
         tc.tile_pool(name="stream", bufs=16) as stream_pool, \
         tc.tile_pool(name="epi", bufs=8) as epi_pool, \
         tc.tile_pool(name="psum_rel", bufs=2, space="PSUM") as psum_rel, \
         tc.tile_pool(name="psum_sm", bufs=2, space="PSUM") as psum_sm:

        # ---------- consts on the scalar HWDGE ring (runs concurrently with
        # the SWDGE stream, which owns the fast queue end to end). Once the
        # SWDGE ring saturates, HWDGE packets starve (~3 GB/s), so both
        # consts must clear in the first ~10 us: tiny pB first (epilogues
        # depend on it), then pA. The sync ring is avoided entirely — a pB
        # left there finished at ~74 us and gated every epilogue.
        pA = const_pool.tile([128, A_W], BF16)
        nc.scalar.dma_start(pA[:], packA_d[:])
        pB = const_pool.tile([128, B_W], F32)
        nc.sync.dma_start(pB[:], packB_d[:])

        # ---------- prologue: u[b,e] with e on partitions, bf16 ----------
        # qpT[m,b] = sum_h W_q[h,m] * q[b,h]
        vT_sb = const_pool.tile([128, MK, BPC], BF16)
        for mk in range(MK):
            qpT_ps = psum_sm.tile([128, BPC], F32)
            for hk in range(HK):
                nc.tensor.matmul(
                    qpT_ps[:],
                    pA[:, A_WQ + hk * MID + mk * 128:A_WQ + hk * MID + (mk + 1) * 128],
                    pA[:, A_Q + hk * BPC:A_Q + (hk + 1) * BPC],
                    start=(hk == 0), stop=(hk == HK - 1),
                )
            # vT[m,b] = (qpT[m,b] + b_q[m]) * W_out[m]
            nc.vector.tensor_scalar(
                out=vT_sb[:, mk, :], in0=qpT_ps[:],
                scalar1=pB[:, B_BW + mk:B_BW + mk + 1],
                scalar2=pB[:, B_BW + MK + mk:B_BW + MK + mk + 1],
                op0=Alu.add, op1=Alu.mult,
            )

        # uT[e', h, b] = sum_m W_r[128h+e', m] * vT[m, b]  (partition = e')
        uTb_sb = const_pool.tile([128, EH, BPC], BF16)
        for h in range(EH):
            uT_ps = psum_sm.tile([128, BPC], F32)
            for mk in range(MK):
                nc.tensor.matmul(
                    uT_ps[:],
                    pA[:, A_WR + mk * E + h * 128:A_WR + mk * E + (h + 1) * 128],
                    vT_sb[:, mk, :],
                    start=(mk == 0), stop=(mk == MK - 1),
                )
            nc.vector.tensor_copy(uTb_sb[:, h, :], uT_ps[:])

        outT_sb = const_pool.tile([N, BPC], F32)

        # ---------- main stream + windowed mat-vec on the PE ----------
        last_part = {}

        def sample(b):
            psum_b = psum_rel.tile([128, N], F32)
            for (i0, ib) in CHUNKS[b]:
                L = N * ib + OVL
                ch = []
                for h in range(EH):
                    t_ = stream_pool.tile([128, L], BF16)
                    # the whole stream stays on the SWDGE queue: a second
                    # (HWDGE) queue caps at ~125 GB/s and drags the SDMA
                    # engines' round-robin down with it (measured 250 vs
                    # 360 GB/s single-queue)
                    eng = nc.gpsimd
                    eng.dma_start(
                        t_[:],
                        transF_d[b, h * 128:(h + 1) * 128, N * i0:N * i0 + L],
                    )
                    ch.append(t_)
                for il in range(ib):
                    t = i0 + il
                    c0 = N * il
                    nc.tensor.matmul(psum_b[0:STAT, t:t + 1],
                                     ch[0][:, c0:c0 + STAT],
                                     uTb_sb[:, 0, b:b + 1],
                                     start=True, stop=False)
                    nc.tensor.matmul(psum_b[0:STAT, t:t + 1],
                                     ch[1][:, c0:c0 + STAT],
                                     uTb_sb[:, 1, b:b + 1],
                                     start=False, stop=True)
                if b == BPC - 1 and i0 + ib == 75:
                    # partial epilogue over cols 0:75 runs while the PE is
                    # still on the taper chunks; only cols 75:100 remain in
                    # the tail
                    P0 = epi_pool.tile([N, N], F32, name="P0_last")
                    P = epi_pool.tile([N, N], F32, name="P_last")
                    S1 = epi_pool.tile([N, 1], F32, name="S1_last")
                    nc.scalar.activation(P0[:, 0:75], psum_b[0:N, 0:75],
                                         ActF.Exp, scale=1.0)
                    nc.vector.scalar_tensor_tensor(
                        out=P[:, 0:75], in0=P0[:, 0:75], scalar=1.0,
                        in1=pB[0:N, B_MASK + b * N:B_MASK + b * N + 75],
                        op0=Alu.mult, op1=Alu.mult, accum_out=S1[:],
                    )
                    last_part.update(P0=P0, P=P, S1=S1)
            return psum_b

        def epilogue_last(b, psum_b):
            P0, P, S1 = last_part["P0"], last_part["P"], last_part["S1"]
            nc.scalar.activation(P0[:, 75:100], psum_b[0:N, 75:100], ActF.Exp,
                                 scale=1.0)
            S2 = epi_pool.tile([N, 1], F32)
            nc.vector.scalar_tensor_tensor(
                out=P[:, 75:100], in0=P0[:, 75:100], scalar=1.0,
                in1=pB[0:N, B_MASK + b * N + 75:B_MASK + (b + 1) * N],
                op0=Alu.mult, op1=Alu.mult, accum_out=S2[:],
            )
            S = epi_pool.tile([N, 1], F32)
            nc.vector.tensor_add(S[:], S1[:], S2[:])
            Sinv = epi_pool.tile([N, 1], F32)
            nc.vector.reciprocal(Sinv[:], S[:])
            w_sb = epi_pool.tile([N, 1], F32)
            nc.vector.tensor_mul(w_sb[:], pB[0:N, B_Z + b:B_Z + b + 1], Sinv[:])
            o_ps = psum_sm.tile([N, 1], F32)
            nc.tensor.matmul(o_ps[:], P[:], w_sb[:], start=True, stop=True)
            nc.scalar.copy(outT_sb[:, b:b + 1], o_ps[:])

        def epilogue(b, psum_b):
            # P0[j,i] = exp(rel[j,i])  (rel bounded ~|6|, no max-shift needed)
            P0 = epi_pool.tile([N, N], F32)
            nc.scalar.activation(P0[:], psum_b[0:N, :], ActF.Exp, scale=1.0)
            # P = P0 * mask; S[j] = sum_i P[j,i]  (one DVE op)
            P = epi_pool.tile([N, N], F32)
            S = epi_pool.tile([N, 1], F32)
            nc.vector.scalar_tensor_tensor(
                out=P[:], in0=P0[:], scalar=1.0,
                in1=pB[0:N, B_MASK + b * N:B_MASK + (b + 1) * N],
                op0=Alu.mult, op1=Alu.mult, accum_out=S[:],
            )
            Sinv = epi_pool.tile([N, 1], F32)
            nc.vector.reciprocal(Sinv[:], S[:])
            w_sb = epi_pool.tile([N, 1], F32)
            nc.vector.tensor_mul(w_sb[:], pB[0:N, B_Z + b:B_Z + b + 1], Sinv[:])
            # out[i] = sum_j P[j,i] * w[j]
            o_ps = psum_sm.tile([N, 1], F32)
            nc.tensor.matmul(o_ps[:], P[:], w_sb[:], start=True, stop=True)
            nc.scalar.copy(outT_sb[:, b:b + 1], o_ps[:])

        # delay each epilogue by one sample: the next sample's stream DVE/ACT
        # ops must not queue behind the epilogue's, or the stream stalls at
        # every sample boundary
        ps_prev = None
        for b in range(BPC):
            psum_b = sample(b)
            if ps_prev is not None:
                epilogue(b - 1, ps_prev)
            ps_prev = psum_b
        epilogue_last(BPC - 1, ps_prev)

        nc.sync.dma_start(outT_d[:], outT_sb[:])

    nc.compile()
    return nc


_nc_cache = None


def _get_nc():
    global _nc_cache
    if _nc_cache is None:
        _nc_cache = _build()
    return _nc_cache


def _make_in_maps(q, trans_mat, r_mask, z_logits, W_r, b_r, W_q, b_q, W_out, b_out):
    bf16 = ml_dtypes.bfloat16
    in_maps = []
    transF = np.zeros((B, E, NF), dtype=bf16)
    transF[:, :, :N * N] = (trans_mat.transpose(0, 3, 1, 2)
                            .reshape(B, E, N * N).astype(bf16))
    Wqpk = W_q.reshape(HK, 128, MID).transpose(1, 0, 2).reshape(128, HK * MID)
    Wrpk = W_r.T.reshape(MK, 128, E).transpose(1, 0, 2).reshape(128, MK * E)
    bw = np.concatenate([b_q.reshape(MK, 128).T, W_out.reshape(MK, 128).T], axis=1)
    for c in range(NCORES):
        b0 = c * BPC
        qpk = (q[b0:b0 + BPC].T.reshape(HK, 128, BPC)
               .transpose(1, 0, 2).reshape(128, HK * BPC))
        packA = np.concatenate([Wqpk, Wrpk, qpk], axis=1).astype(bf16)
        packB = np.zeros((128, B_W), dtype=np.float32)
        packB[:, B_BW:B_BW + 2 * MK] = bw
        packB[0:N, B_Z:B_Z + BPC] = z_logits[b0:b0 + BPC].T
        packB[0:N, B_MASK:] = (
            r_mask[b0:b0 + BPC].transpose(2, 0, 1).reshape(N, BPC * N)
            .astype(np.float32))
        in_maps.append({
            "transF": np.ascontiguousarray(transF[b0:b0 + BPC]),
            "packA": np.ascontiguousarray(packA),
            "packB": packB,
        })
    return in_maps


def _run(inputs, trace=False, **kwargs):
    nc = _get_nc()
    in_maps = _make_in_maps(**inputs)
    res = run_bass_kernel_spmd(nc, in_maps, list(range(NCORES)),
                               trace=trace, **kwargs)
    out = np.empty((B, N), dtype=np.float32)
    for c in range(NCORES):
        out[c * BPC:(c + 1) * BPC, :] = np.asarray(res.results[c]["outT"]).T
    return out, res


def kernel(**inputs):
    out, _ = _run(inputs)
    return out


# revision 25
# speedup vs baseline: 1.0161x; 1.0161x over previous
"""Trainium2 Bass kernel for nn_Net_76270029242478 (gnn_message_passing).

Math (B=32, N=100, E=256, H=1024, MID=256):
  t        = einsum('bije,em->bijm', trans_mat, W_r) + b_r
  qp       = q @ W_q + b_q
  relation = einsum('bijm,m->bij', t * qp[:,None,None,:], W_out[:,0]) + b_out
  relation = where(r_mask==0, -inf, relation); softmax over i (axis=1)
  out      = einsum('bij,bj->bi', softmax, z_logits)

Algebraic fold (exact):
  relation[b,i,j] = trans_mat[b,i,j,:] . u[b,:] + c[b]
    u[b,e] = sum_m W_r[e,m] * (qp[b,m]+b_q[m]) * W_out[m,0]
  c[b] is constant over (i,j) so it cancels in the softmax over i.

Device strategy (v2 — bf16 stream + overlapped 128-col stationary windows):
  - The device matmul consumes bf16 anyway, so the host casts trans to bf16
    before staging: the HBM stream halves to 20.5 MB/core (~57 us at the
    ~358 GB/s per-core HBM roofline) vs 41 MB f32, and the on-device
    DVE/ACT cast pipeline disappears entirely.
  - Host lays trans out flat as [b, e, i*N+j] (j contiguous), zero-padded
    to NF columns, so every DMA is one fat contiguous run per partition.
  - The PE mat-vec rate in the f32 baseline was LDWEIGHTS-bound: 100-col
    stationary loads stream at 1 col / 1.2 GHz = 83 ns, 87 ns/pair
    measured, 70 us total — above the new DMA floor. Fix: stationary
    windows of 128 columns taken at stride 100 from the flat buffer
    (row i's 100 j-columns + 28 duplicate elements of row i+1). 128-col
    bf16 loads are eligible for Fast Weight Load (~2x), and psum
    partition p<100 of column t still receives exactly rel[j=p, i=t];
    partitions 100:128 hold duplicates that are never read.
  - Each DMA chunk carries 28 duplicate trailing columns so stationary
    windows never cross chunk-tile boundaries (+0.6% bytes).
  - The whole stream runs on the SWDGE (gpsimd) queue: splitting across
    a second HWDGE queue measured 250 GB/s aggregate vs ~420 single-queue.
    Consts ride the scalar HWDGE ring and must clear before the SWDGE
    queue saturates (starved HWDGE packets trickle at ~3 GB/s after that).
  - Softmax lands in [j_part, i_free] layout: exp (ACT), mask-mult +
    denominator (one DVE op with accum), final aggregation is one matmul.
  - The last sample's chunks taper so the post-stream drain is short.

Sharding: data-parallel over batch, 4 samples per core x 8 cores.
"""

import ml_dtypes
import numpy as np

import concourse.bass as bass
import concourse.tile as tile
from concourse import bacc, mybir
from concourse.bass_utils import run_bass_kernel_spmd

F32 = mybir.dt.float32
BF16 = mybir.dt.bfloat16
Alu = mybir.AluOpType
ActF = mybir.ActivationFunctionType

B, N, E, H, MID = 32, 100, 256, 1024, 256
NCORES = 8
BPC = B // NCORES       # samples per core = 4
EH = E // 128           # 2 e-halves (contraction chunks)
HK = H // 128           # 8 contraction chunks for q @ W_q
MK = MID // 128         # 2 contraction chunks
STAT = 128              # stationary window width (128 -> FWL-eligible)
OVL = STAT - N          # overlap columns carried past each chunk end
NF = N * N + 48         # flat (i,j) length, zero-padded
# i-row chunk schedule per sample: (row0, nrows). ~1.3 MB transfers keep
# the SWDGE descriptor pipeline dense (whole-sample 2.6 MB transfers delay
# the doorbell and sample-serialize the PE); sample 0 starts small so the
# PE gets work early, the last sample tapers so the post-stream drain is
# short.
CHUNKS = {
    0: [(0, 8), (8, 42), (50, 50)],
    1: [(0, 50), (50, 50)],
    2: [(0, 50), (50, 50)],
    3: [(0, 50), (50, 25), (75, 13), (88, 6), (94, 6)],
}
EPI_SPLIT = 88          # last sample: cols 0:88 epilogued early, 88:100 in tail
# packA (bf16) column offsets
A_WQ, A_WR, A_Q = 0, HK * MID, HK * MID + MK * E
A_W = A_Q + HK * BPC                     # 2592
# packB (f32) column offsets
B_BW, B_Z, B_MASK = 0, 2 * MK, 2 * MK + BPC
B_W = B_MASK + BPC * N                   # 408


def _build():
    nc = bacc.Bacc("TRN2", target_bir_lowering=False, debug=False,
                   num_devices=NCORES)

    # trans pre-flattened on host to [b, e, i*N+j] bf16 (e on partitions)
    transF_d = nc.declare_dram_parameter("transF", [BPC, E, NF], BF16,
                                         isOutput=False)
    packA_d = nc.declare_dram_parameter("packA", [128, A_W], BF16, isOutput=False)
    packB_d = nc.declare_dram_parameter("packB", [128, B_W], F32, isOutput=False)
    outT_d = nc.declare_dram_parameter("outT", [N, BPC], F32, isOutput=True)

    with tile.TileContext(nc) as tc, \
         tc.tile_pool(name="const", bufs=1) as const_pool, \
         tc.tile_pool(name="stream", bufs=16) as stream_pool, \
         tc.tile_pool(name="epi", bufs=8) as epi_pool, \
         tc.tile_pool(name="psum_rel", bufs=2, space="PSUM") as psum_rel, \
         tc.tile_pool(name="psum_sm", bufs=2, space="PSUM") as psum_sm:

        # ---------- consts on the scalar HWDGE ring (runs concurrently with
        # the SWDGE stream, which owns the fast queue end to end). Once the
        # SWDGE ring saturates, HWDGE packets starve (~3 GB/s), so both
        # consts must clear in the first ~10 us: tiny pB first (epilogues
        # depend on it), then pA. The sync ring is avoided entirely — a pB
        # left there finished at ~74 us and gated every epilogue.
        pA = const_pool.tile([128, A_W], BF16)
        nc.scalar.dma_start(pA[:], packA_d[:])
        pB = const_pool.tile([128, B_W], F32)
        nc.sync.dma_start(pB[:], packB_d[:])

        # ---------- prologue: u[b,e] with e on partitions, bf16 ----------
        # qpT[m,b] = sum_h W_q[h,m] * q[b,h]
        vT_sb = const_pool.tile([128, MK, BPC], BF16)
        for mk in range(MK):
            qpT_ps = psum_sm.tile([128, BPC], F32)
            for hk in range(HK):
                nc.tensor.matmul(
                    qpT_ps[:],
                    pA[:, A_WQ + hk * MID + mk * 128:A_WQ + hk * MID + (mk + 1) * 128],
                    pA[:, A_Q + hk * BPC:A_Q + (hk + 1) * BPC],
                    start=(hk == 0), stop=(hk == HK - 1),
                )
            # vT[m,b] = (qpT[m,b] + b_q[m]) * W_out[m]
            nc.vector.tensor_scalar(
                out=vT_sb[:, mk, :], in0=qpT_ps[:],
                scalar1=pB[:, B_BW + mk:B_BW + mk + 1],
                scalar2=pB[:, B_BW + MK + mk:B_BW + MK + mk + 1],
                op0=Alu.add, op1=Alu.mult,
            )

        # uT[e', h, b] = sum_m W_r[128h+e', m] * vT[m, b]  (partition = e')
        uTb_sb = const_pool.tile([128, EH, BPC], BF16)
        for h in range(EH):
            uT_ps = psum_sm.tile([128, BPC], F32)
            for mk in range(MK):
                nc.tensor.matmul(
                    uT_ps[:],
                    pA[:, A_WR + mk * E + h * 128:A_WR + mk * E + (h + 1) * 128],
                    vT_sb[:, mk, :],
                    start=(mk == 0), stop=(mk == MK - 1),
                )
            nc.vector.tensor_copy(uTb_sb[:, h, :], uT_ps[:])

        outT_sb = const_pool.tile([N, BPC], F32)

        # ---------- main stream + windowed mat-vec on the PE ----------
        last_part = {}

        def sample(b):
            psum_b = psum_rel.tile([128, N], F32)
            for (i0, ib) in CHUNKS[b]:
                L = N * ib + OVL
                ch = []
                for h in range(EH):
                    t_ = stream_pool.tile([128, L], BF16)
                    # the whole stream stays on the SWDGE queue: a second
                    # (HWDGE) queue caps at ~125 GB/s and drags the SDMA
                    # engines' round-robin down with it (measured 250 vs
                    # 360 GB/s single-queue)
                    eng = nc.gpsimd
                    eng.dma_start(
                        t_[:],
                        transF_d[b, h * 128:(h + 1) * 128, N * i0:N * i0 + L],
                    )
                    ch.append(t_)
                for il in range(ib):
                    t = i0 + il
                    c0 = N * il
                    nc.tensor.matmul(psum_b[0:STAT, t:t + 1],
                                     ch[0][:, c0:c0 + STAT],
                                     uTb_sb[:, 0, b:b + 1],
                                     start=True, stop=False)
                    nc.tensor.matmul(psum_b[0:STAT, t:t + 1],
                                     ch[1][:, c0:c0 + STAT],
                                     uTb_sb[:, 1, b:b + 1],
                                     start=False, stop=True)
                if b == BPC - 1 and i0 + ib == EPI_SPLIT:
                    # partial epilogue over cols 0:EPI_SPLIT runs while the
                    # PE is still on the taper chunks; only the last 12
                    # columns remain in the tail
                    P0 = epi_pool.tile([N, N], F32, name="P0_last")
                    P = epi_pool.tile([N, N], F32, name="P_last")
                    S1 = epi_pool.tile([N, 1], F32, name="S1_last")
                    nc.scalar.activation(P0[:, 0:EPI_SPLIT],
                                         psum_b[0:N, 0:EPI_SPLIT],
                                         ActF.Exp, scale=1.0)
                    nc.vector.scalar_tensor_tensor(
                        out=P[:, 0:EPI_SPLIT], in0=P0[:, 0:EPI_SPLIT],
                        scalar=1.0,
                        in1=pB[0:N, B_MASK + b * N:B_MASK + b * N + EPI_SPLIT],
                        op0=Alu.mult, op1=Alu.mult, accum_out=S1[:],
                    )
                    last_part.update(P0=P0, P=P, S1=S1)
            return psum_b

        def epilogue_last(b, psum_b):
            P0, P, S1 = last_part["P0"], last_part["P"], last_part["S1"]
            nc.scalar.activation(P0[:, EPI_SPLIT:N], psum_b[0:N, EPI_SPLIT:N],
                                 ActF.Exp, scale=1.0)
            S2 = epi_pool.tile([N, 1], F32)
            nc.vector.scalar_tensor_tensor(
                out=P[:, EPI_SPLIT:N], in0=P0[:, EPI_SPLIT:N], scalar=1.0,
                in1=pB[0:N, B_MASK + b * N + EPI_SPLIT:B_MASK + (b + 1) * N],
                op0=Alu.mult, op1=Alu.mult, accum_out=S2[:],
            )
            S = epi_pool.tile([N, 1], F32)
            nc.vector.tensor_add(S[:], S1[:], S2[:])
            Sinv = epi_pool.tile([N, 1], F32)
            nc.vector.reciprocal(Sinv[:], S[:])
            w_sb = epi_pool.tile([N, 1], F32)
            nc.vector.tensor_mul(w_sb[:], pB[0:N, B_Z + b:B_Z + b + 1], Sinv[:])
            o_ps = psum_sm.tile([N, 1], F32)
            nc.tensor.matmul(o_ps[:], P[:], w_sb[:], start=True, stop=True)
            nc.scalar.copy(outT_sb[:, b:b + 1], o_ps[:])

        def epilogue(b, psum_b):
            # P0[j,i] = exp(rel[j,i])  (rel bounded ~|6|, no max-shift needed)
            P0 = epi_pool.tile([N, N], F32)
            nc.scalar.activation(P0[:], psum_b[0:N, :], ActF.Exp, scale=1.0)
            # P = P0 * mask; S[j] = sum_i P[j,i]  (one DVE op)
            P = epi_pool.tile([N, N], F32)
            S = epi_pool.tile([N, 1], F32)
            nc.vector.scalar_tensor_tensor(
                out=P[:], in0=P0[:], scalar=1.0,
                in1=pB[0:N, B_MASK + b * N:B_MASK + (b + 1) * N],
                op0=Alu.mult, op1=Alu.mult, accum_out=S[:],
            )
            Sinv = epi_pool.tile([N, 1], F32)
            nc.vector.reciprocal(Sinv[:], S[:])
            w_sb = epi_pool.tile([N, 1], F32)
            nc.vector.tensor_mul(w_sb[:], pB[0:N, B_Z + b:B_Z + b + 1], Sinv[:])
            # out[i] = sum_j P[j,i] * w[j]
            o_ps = psum_sm.tile([N, 1], F32)
            nc.tensor.matmul(o_ps[:], P[:], w_sb[:], start=True, stop=True)
            nc.scalar.copy(outT_sb[:, b:b + 1], o_ps[:])

        # delay each epilogue by one sample: the next sample's stream DVE/ACT
        # ops must not queue behind the epilogue's, or the stream stalls at
        # every sample boundary
        ps_prev = None
        for b in range(BPC):
            psum_b = sample(b)
            if ps_prev is not None:
                epilogue(b - 1, ps_prev)
            ps_prev = psum_b
        epilogue_last(BPC - 1, ps_prev)

        nc.sync.dma_start(outT_d[:], outT_sb[:])

    nc.compile()
    return nc


_nc_cache = None


def _get_nc():
    global _nc_cache
    if _nc_cache is None:
        _nc_cache = _build()
    return _nc_cache


def _make_in_maps(q, trans_mat, r_mask, z_logits, W_r, b_r, W_q, b_q, W_out, b_out):
    bf16 = ml_dtypes.bfloat16
    in_maps = []
    transF = np.zeros((B, E, NF), dtype=bf16)
    transF[:, :, :N * N] = (trans_mat.transpose(0, 3, 1, 2)
                            .reshape(B, E, N * N).astype(bf16))
    Wqpk = W_q.reshape(HK, 128, MID).transpose(1, 0, 2).reshape(128, HK * MID)
    Wrpk = W_r.T.reshape(MK, 128, E).transpose(1, 0, 2).reshape(128, MK * E)
    bw = np.concatenate([b_q.reshape(MK, 128).T, W_out.reshape(MK, 128).T], axis=1)
    for c in range(NCORES):
        b0 = c * BPC
        qpk = (q[b0:b0 + BPC].T.reshape(HK, 128, BPC)
               .transpose(1, 0, 2).reshape(128, HK * BPC))
        packA = np.concatenate([Wqpk, Wrpk, qpk], axis=1).astype(bf16)
        packB = np.zeros((128, B_W), dtype=np.float32)
        packB[:, B_BW:B_BW + 2 * MK] = bw
        packB[0:N, B_Z:B_Z + BPC] = z_logits[b0:b0 + BPC].T
        packB[0:N, B_MASK:] = (
            r_mask[b0:b0 + BPC].transpose(2, 0, 1).reshape(N, BPC * N)
            .astype(np.float32))
        in_maps.append({
            "transF": np.ascontiguousarray(transF[b0:b0 + BPC]),
            "packA": np.ascontiguousarray(packA),
            "packB": packB,
        })
    return in_maps


def _run(inputs, trace=False, **kwargs):
    nc = _get_nc()
    in_maps = _make_in_maps(**inputs)
    res = run_bass_kernel_spmd(nc, in_maps, list(range(NCORES)),
                               trace=trace, **kwargs)
    out = np.empty((B, N), dtype=np.float32)
    for c in range(NCORES):
        out[c * BPC:(c + 1) * BPC, :] = np.asarray(res.results[c]["outT"]).T
    return out, res


def kernel(**inputs):
    out, _ = _run(inputs)
    return out


# revision 26
# speedup vs baseline: 1.0584x; 1.0416x over previous
"""Trainium2 Bass kernel for nn_Net_76270029242478 (gnn_message_passing).

Math (B=32, N=100, E=256, H=1024, MID=256):
  t        = einsum('bije,em->bijm', trans_mat, W_r) + b_r
  qp       = q @ W_q + b_q
  relation = einsum('bijm,m->bij', t * qp[:,None,None,:], W_out[:,0]) + b_out
  relation = where(r_mask==0, -inf, relation); softmax over i (axis=1)
  out      = einsum('bij,bj->bi', softmax, z_logits)

Algebraic fold (exact):
  relation[b,i,j] = trans_mat[b,i,j,:] . u[b,:] + c[b]
    u[b,e] = sum_m W_r[e,m] * (qp[b,m]+b_q[m]) * W_out[m,0]
  c[b] is constant over (i,j) so it cancels in the softmax over i.

Device strategy (v2 — bf16 stream + overlapped 128-col stationary windows):
  - The device matmul consumes bf16 anyway, so the host casts trans to bf16
    before staging: the HBM stream halves to 20.5 MB/core (~57 us at the
    ~358 GB/s per-core HBM roofline) vs 41 MB f32, and the on-device
    DVE/ACT cast pipeline disappears entirely.
  - Host lays trans out flat as [b, e, i*N+j] (j contiguous), zero-padded
    to NF columns, so every DMA is one fat contiguous run per partition.
  - The PE mat-vec rate in the f32 baseline was LDWEIGHTS-bound: 100-col
    stationary loads stream at 1 col / 1.2 GHz = 83 ns, 87 ns/pair
    measured, 70 us total — above the new DMA floor. Fix: stationary
    windows of 128 columns taken at stride 100 from the flat buffer
    (row i's 100 j-columns + 28 duplicate elements of row i+1). 128-col
    bf16 loads are eligible for Fast Weight Load (~2x), and psum
    partition p<100 of column t still receives exactly rel[j=p, i=t];
    partitions 100:128 hold duplicates that are never read.
  - Each DMA chunk carries 28 duplicate trailing columns so stationary
    windows never cross chunk-tile boundaries (+0.6% bytes).
  - The whole stream runs on the SWDGE (gpsimd) queue: splitting across
    a second HWDGE queue measured 250 GB/s aggregate vs ~420 single-queue.
    Consts ride the scalar HWDGE ring and must clear before the SWDGE
    queue saturates (starved HWDGE packets trickle at ~3 GB/s after that).
  - Softmax lands in [j_part, i_free] layout: exp (ACT), mask-mult +
    denominator (one DVE op with accum), final aggregation is one matmul.
  - The last sample's chunks taper so the post-stream drain is short.

Sharding: data-parallel over batch, 4 samples per core x 8 cores.
"""

import ml_dtypes
import numpy as np

import concourse.bass as bass
import concourse.tile as tile
from concourse import bacc, mybir
from concourse.bass_utils import run_bass_kernel_spmd

F32 = mybir.dt.float32
BF16 = mybir.dt.bfloat16
Alu = mybir.AluOpType
ActF = mybir.ActivationFunctionType

B, N, E, H, MID = 32, 100, 256, 1024, 256
NCORES = 8
BPC = B // NCORES       # samples per core = 4
EH = E // 128           # 2 e-halves (contraction chunks)
HK = H // 128           # 8 contraction chunks for q @ W_q
MK = MID // 128         # 2 contraction chunks
STAT = 128              # stationary window width (128 -> FWL-eligible)
OVL = STAT - N          # overlap columns carried past each chunk end
NF = N * N + 48         # flat (i,j) length, zero-padded
# i-row chunk schedule per sample: (row0, nrows). ~1.3 MB transfers keep
# the SWDGE descriptor pipeline dense (whole-sample 2.6 MB transfers delay
# the doorbell and sample-serialize the PE); sample 0 starts small so the
# PE gets work early, the last sample tapers so the post-stream drain is
# short.
CHUNKS = {
    0: [(0, 8), (8, 42), (50, 50)],
    1: [(0, 50), (50, 50)],
    2: [(0, 50), (50, 50)],
    3: [(0, 50), (50, 25), (75, 13), (88, 6), (94, 6)],
}
EPI_SPLIT = 88          # last sample: cols 0:88 epilogued early, 88:100 in tail
# packA (bf16) column offsets
A_WQ, A_WR, A_Q = 0, HK * MID, HK * MID + MK * E
A_W = A_Q + HK * BPC                     # 2592
# packB (f32) column offsets
B_BW, B_Z, B_MASK = 0, 2 * MK, 2 * MK + BPC
B_W = B_MASK + BPC * N                   # 408


def _build():
    nc = bacc.Bacc("TRN2", target_bir_lowering=False, debug=False,
                   num_devices=NCORES)

    # trans pre-flattened on host to [b, e, i*N+j] bf16 (e on partitions)
    transF_d = nc.declare_dram_parameter("transF", [BPC, E, NF], BF16,
                                         isOutput=False)
    packA_d = nc.declare_dram_parameter("packA", [128, A_W], BF16, isOutput=False)
    packB_d = nc.declare_dram_parameter("packB", [128, B_W], F32, isOutput=False)
    outT_d = nc.declare_dram_parameter("outT", [N, BPC], F32, isOutput=True)

    with tile.TileContext(nc) as tc, \
         tc.tile_pool(name="const", bufs=1) as const_pool, \
         tc.tile_pool(name="stream", bufs=16) as stream_pool, \
         tc.tile_pool(name="epi", bufs=8) as epi_pool, \
         tc.tile_pool(name="psum_rel", bufs=2, space="PSUM") as psum_rel, \
         tc.tile_pool(name="psum_sm", bufs=2, space="PSUM") as psum_sm:

        # ---------- consts on the scalar HWDGE ring (runs concurrently with
        # the SWDGE stream, which owns the fast queue end to end). Once the
        # SWDGE ring saturates, HWDGE packets starve (~3 GB/s), so both
        # consts must clear in the first ~10 us: tiny pB first (epilogues
        # depend on it), then pA. The sync ring is avoided entirely — a pB
        # left there finished at ~74 us and gated every epilogue.
        pA = const_pool.tile([128, A_W], BF16)
        nc.scalar.dma_start(pA[:], packA_d[:])
        pB = const_pool.tile([128, B_W], F32)
        nc.sync.dma_start(pB[:], packB_d[:])

        # ---------- prologue: u[b,e] with e on partitions, bf16 ----------
        # qpT[m,b] = sum_h W_q[h,m] * q[b,h]
        vT_sb = const_pool.tile([128, MK, BPC], BF16)
        for mk in range(MK):
            qpT_ps = psum_sm.tile([128, BPC], F32)
            for hk in range(HK):
                nc.tensor.matmul(
                    qpT_ps[:],
                    pA[:, A_WQ + hk * MID + mk * 128:A_WQ + hk * MID + (mk + 1) * 128],
                    pA[:, A_Q + hk * BPC:A_Q + (hk + 1) * BPC],
                    start=(hk == 0), stop=(hk == HK - 1),
                )
            # vT[m,b] = (qpT[m,b] + b_q[m]) * W_out[m]
            nc.vector.tensor_scalar(
                out=vT_sb[:, mk, :], in0=qpT_ps[:],
                scalar1=pB[:, B_BW + mk:B_BW + mk + 1],
                scalar2=pB[:, B_BW + MK + mk:B_BW + MK + mk + 1],
                op0=Alu.add, op1=Alu.mult,
            )

        # uT[e', h, b] = sum_m W_r[128h+e', m] * vT[m, b]  (partition = e')
        uTb_sb = const_pool.tile([128, EH, BPC], BF16)
        for h in range(EH):
            uT_ps = psum_sm.tile([128, BPC], F32)
            for mk in range(MK):
                nc.tensor.matmul(
                    uT_ps[:],
                    pA[:, A_WR + mk * E + h * 128:A_WR + mk * E + (h + 1) * 128],
                    vT_sb[:, mk, :],
                    start=(mk == 0), stop=(mk == MK - 1),
                )
            nc.vector.tensor_copy(uTb_sb[:, h, :], uT_ps[:])

        outT_sb = const_pool.tile([N, BPC], F32)

        # ---------- main stream + windowed mat-vec on the PE ----------
        last_part = {}

        def sample(b):
            psum_b = psum_rel.tile([128, N], F32)
            for ci, (i0, ib) in enumerate(CHUNKS[b]):
                L = N * ib + OVL
                ch = []
                for h in range(EH):
                    t_ = stream_pool.tile([128, L], BF16)
                    # the steady stream stays on the SWDGE queue: a second
                    # (HWDGE) queue caps at ~125 GB/s and drags the SDMA
                    # engines' round-robin down with it (measured 250 vs
                    # 420 GB/s single-queue). Exception: the very first
                    # chunk pair rides the sync HWDGE ring, which is idle
                    # and fast before the SWDGE queue saturates — those
                    # bytes come straight off the laggard engine's budget.
                    eng = nc.sync if (b == 0 and ci == 0) else nc.gpsimd
                    eng.dma_start(
                        t_[:],
                        transF_d[b, h * 128:(h + 1) * 128, N * i0:N * i0 + L],
                    )
                    ch.append(t_)
                for il in range(ib):
                    t = i0 + il
                    c0 = N * il
                    nc.tensor.matmul(psum_b[0:STAT, t:t + 1],
                                     ch[0][:, c0:c0 + STAT],
                                     uTb_sb[:, 0, b:b + 1],
                                     start=True, stop=False)
                    nc.tensor.matmul(psum_b[0:STAT, t:t + 1],
                                     ch[1][:, c0:c0 + STAT],
                                     uTb_sb[:, 1, b:b + 1],
                                     start=False, stop=True)
                if b == BPC - 1 and i0 + ib == EPI_SPLIT:
                    # partial epilogue over cols 0:EPI_SPLIT runs while the
                    # PE is still on the taper chunks; only the last 12
                    # columns remain in the tail
                    P0 = epi_pool.tile([N, N], F32, name="P0_last")
                    P = epi_pool.tile([N, N], F32, name="P_last")
                    S1 = epi_pool.tile([N, 1], F32, name="S1_last")
                    nc.scalar.activation(P0[:, 0:EPI_SPLIT],
                                         psum_b[0:N, 0:EPI_SPLIT],
                                         ActF.Exp, scale=1.0)
                    nc.vector.scalar_tensor_tensor(
                        out=P[:, 0:EPI_SPLIT], in0=P0[:, 0:EPI_SPLIT],
                        scalar=1.0,
                        in1=pB[0:N, B_MASK + b * N:B_MASK + b * N + EPI_SPLIT],
                        op0=Alu.mult, op1=Alu.mult, accum_out=S1[:],
                    )
                    last_part.update(P0=P0, P=P, S1=S1)
            return psum_b

        def epilogue_last(b, psum_b):
            P0, P, S1 = last_part["P0"], last_part["P"], last_part["S1"]
            nc.scalar.activation(P0[:, EPI_SPLIT:N], psum_b[0:N, EPI_SPLIT:N],
                                 ActF.Exp, scale=1.0)
            S2 = epi_pool.tile([N, 1], F32)
            nc.vector.scalar_tensor_tensor(
                out=P[:, EPI_SPLIT:N], in0=P0[:, EPI_SPLIT:N], scalar=1.0,
                in1=pB[0:N, B_MASK + b * N + EPI_SPLIT:B_MASK + (b + 1) * N],
                op0=Alu.mult, op1=Alu.mult, accum_out=S2[:],
            )
            S = epi_pool.tile([N, 1], F32)
            nc.vector.tensor_add(S[:], S1[:], S2[:])
            Sinv = epi_pool.tile([N, 1], F32)
            nc.vector.reciprocal(Sinv[:], S[:])
            w_sb = epi_pool.tile([N, 1], F32)
            nc.vector.tensor_mul(w_sb[:], pB[0:N, B_Z + b:B_Z + b + 1], Sinv[:])
            o_ps = psum_sm.tile([N, 1], F32)
            nc.tensor.matmul(o_ps[:], P[:], w_sb[:], start=True, stop=True)
            nc.scalar.copy(outT_sb[:, b:b + 1], o_ps[:])

        def epilogue(b, psum_b):
            # P0[j,i] = exp(rel[j,i])  (rel bounded ~|6|, no max-shift needed)
            P0 = epi_pool.tile([N, N], F32)
            nc.scalar.activation(P0[:], psum_b[0:N, :], ActF.Exp, scale=1.0)
            # P = P0 * mask; S[j] = sum_i P[j,i]  (one DVE op)
            P = epi_pool.tile([N, N], F32)
            S = epi_pool.tile([N, 1], F32)
            nc.vector.scalar_tensor_tensor(
                out=P[:], in0=P0[:], scalar=1.0,
                in1=pB[0:N, B_MASK + b * N:B_MASK + (b + 1) * N],
                op0=Alu.mult, op1=Alu.mult, accum_out=S[:],
            )
            Sinv = epi_pool.tile([N, 1], F32)
            nc.vector.reciprocal(Sinv[:], S[:])
            w_sb = epi_pool.tile([N, 1], F32)
            nc.vector.tensor_mul(w_sb[:], pB[0:N, B_Z + b:B_Z + b + 1], Sinv[:])
            # out[i] = sum_j P[j,i] * w[j]
            o_ps = psum_sm.tile([N, 1], F32)
            nc.tensor.matmul(o_ps[:], P[:], w_sb[:], start=True, stop=True)
            nc.scalar.copy(outT_sb[:, b:b + 1], o_ps[:])

        # delay each epilogue by one sample: the next sample's stream DVE/ACT
        # ops must not queue behind the epilogue's, or the stream stalls at
        # every sample boundary
        ps_prev = None
        for b in range(BPC):
            psum_b = sample(b)
            if ps_prev is not None:
                epilogue(b - 1, ps_prev)
            ps_prev = psum_b
        epilogue_last(BPC - 1, ps_prev)

        nc.sync.dma_start(outT_d[:], outT_sb[:])

    nc.compile()
    return nc


_nc_cache = None


def _get_nc():
    global _nc_cache
    if _nc_cache is None:
        _nc_cache = _build()
    return _nc_cache


def _make_in_maps(q, trans_mat, r_mask, z_logits, W_r, b_r, W_q, b_q, W_out, b_out):
    bf16 = ml_dtypes.bfloat16
    in_maps = []
    transF = np.zeros((B, E, NF), dtype=bf16)
    transF[:, :, :N * N] = (trans_mat.transpose(0, 3, 1, 2)
                            .reshape(B, E, N * N).astype(bf16))
    Wqpk = W_q.reshape(HK, 128, MID).transpose(1, 0, 2).reshape(128, HK * MID)
    Wrpk = W_r.T.reshape(MK, 128, E).transpose(1, 0, 2).reshape(128, MK * E)
    bw = np.concatenate([b_q.reshape(MK, 128).T, W_out.reshape(MK, 128).T], axis=1)
    for c in range(NCORES):
        b0 = c * BPC
        qpk = (q[b0:b0 + BPC].T.reshape(HK, 128, BPC)
               .transpose(1, 0, 2).reshape(128, HK * BPC))
        packA = np.concatenate([Wqpk, Wrpk, qpk], axis=1).astype(bf16)
        packB = np.zeros((128, B_W), dtype=np.float32)
        packB[:, B_BW:B_BW + 2 * MK] = bw
        packB[0:N, B_Z:B_Z + BPC] = z_logits[b0:b0 + BPC].T
        packB[0:N, B_MASK:] = (
            r_mask[b0:b0 + BPC].transpose(2, 0, 1).reshape(N, BPC * N)
            .astype(np.float32))
        in_maps.append({
            "transF": np.ascontiguousarray(transF[b0:b0 + BPC]),
            "packA": np.ascontiguousarray(packA),
            "packB": packB,
        })
    return in_maps


def _run(inputs, trace=False, **kwargs):
    nc = _get_nc()
    in_maps = _make_in_maps(**inputs)
    res = run_bass_kernel_spmd(nc, in_maps, list(range(NCORES)),
                               trace=trace, **kwargs)
    out = np.empty((B, N), dtype=np.float32)
    for c in range(NCORES):
        out[c * BPC:(c + 1) * BPC, :] = np.asarray(res.results[c]["outT"]).T
    return out, res


def kernel(**inputs):
    out, _ = _run(inputs)
    return out
